# revision 8
# baseline (speedup 1.0000x reference)
"""BilateralGrid (HDRNet slicing) Trainium2 Bass kernel — PE-accumulate edition.

Full inputs -> full output. Sharding: 8 cores = (batch b, H-half); each core
processes an image slab (3, 512, 1024) of one batch.

Per 128-row block the slicing is computed as:

  uz      = 15 * luminance(R, G, B)                       (DVE, f32)
  mz[z]   = relu(1 - |uz - z|)            z = 0..15       (ACT tents, fp16)
  P0/P1   = mz * static x-blend parity profiles           (DVE TT, fp16)
  acc_c  += diag(t[c,z,xs]) @ P_par(xs)[:, window(xs)]    (PE matmuls, PSUM f32)
  acc16_c = copy(acc_c)                                   (ACT, PSUM->SBUF fp16)
  out_o   = clip(acc_{3o}*R + acc_{3o+1}*G + acc_{3o+2}*B + acc_{9+o}, 0, 1)

The x interpolation is folded into the matmul stream: every output column lies
in exactly one x-interval i (between grid columns i and i+1) and receives
tap-A weight (1-fx) toward xs=i and tap-B weight fx toward xs=i+1.  Writing
the tap weights into two static parity profiles (prof0 holds (1-fx) on even
intervals / fx on odd, prof1 the complement) makes the two taps sharing the
per-partition table scalar t[c,z,xs] — tap-A of interval xs and tap-B of
interval xs-1 — occupy ONE contiguous column window with ONE parity tile, so
each (c,z,xs) is a single PSUM-accumulating matmul with stationary
diag(t[c,z,xs]).  Diagonals are built on the fly from a fp16 identity via
one tensor_scalar_mul each.  t[row, c, z, xs] is the y-interpolated grid
table per image row, built on host from the tiny grid input.
"""

import numpy as np

B, C, H, W = 4, 3, 1024, 1024
GD, GH, GW, GC = 16, 16, 8, 12  # grid z, y, x extents; coeff channels
NCORES = 8
ROWS = H // 2  # rows per core
NBLK = ROWS // 128
NI = GW - 1  # 7 x-intervals

_POOL_DIAG_MOD = 3  # xs % _POOL_DIAG_MOD == 0 -> diag built on gpsimd


def _intervals():
    """Interval bounds s_i..e_i (cols with min(floor(ux),6)==i) and fx."""
    ux = np.arange(W) * (GW - 1) / (W - 1.0)
    x0 = np.minimum(np.floor(ux).astype(np.int64), NI - 1)
    bounds = []
    for i in range(NI):
        idx = np.nonzero(x0 == i)[0]
        bounds.append((int(idx[0]), int(idx[-1]) + 1))
    fx = (ux - x0).astype(np.float32)  # in [0,1]; ==1 only at col W-1
    return fx, x0, bounds


_FX, _X0, _BOUNDS = _intervals()


def _window(xs):
    """Column window of the single matmul for tap-pair xs: union of tap-B on
    interval xs-1 and tap-A on interval xs."""
    wa = _BOUNDS[xs - 1][0] if xs > 0 else _BOUNDS[0][0]
    wb = _BOUNDS[xs][1] if xs < NI else _BOUNDS[NI - 1][1]
    return wa, wb


def _host_profiles():
    """prof0/prof1 [128, W] fp16: parity-interleaved (1-fx)/fx profiles."""
    p0 = np.where(_X0 % 2 == 0, 1.0 - _FX, _FX).astype(np.float16)
    p1 = np.where(_X0 % 2 == 0, _FX, 1.0 - _FX).astype(np.float16)
    return (
        np.ascontiguousarray(np.tile(p0, (128, 1))),
        np.ascontiguousarray(np.tile(p1, (128, 1))),
    )


def _host_tables(grid_b, half):
    """t[row, c, z, xs] for this core's 512 rows -> (NBLK, 128, 1536) f32."""
    h = half * ROWS + np.arange(ROWS)
    uy = h * (GH - 1) / (H - 1.0)
    y0 = np.minimum(np.floor(uy).astype(np.int64), GH - 2)
    fy = (uy - y0).astype(np.float32)
    gy0 = grid_b[:, :, y0, :]  # (12, 16, 512, 8)
    gy1 = grid_b[:, :, y0 + 1, :]
    tbl = (1 - fy)[None, None, :, None] * gy0 + fy[None, None, :, None] * gy1
    tbl = np.transpose(tbl, (2, 0, 1, 3))  # (512, c, z, xs)
    return np.ascontiguousarray(
        tbl.reshape(NBLK, 128, GC * GD * GW).astype(np.float32)
    )


def _host_zbias():
    """bias column per z: -z, replicated over partitions -> (128, 16)."""
    return np.tile(-np.arange(GD, dtype=np.float32), (128, 1))


def _host_identity():
    return np.eye(128, dtype=np.float16)


# ---------------------------------------------------------------------------
# Bass program
# ---------------------------------------------------------------------------

_MAX_WAITS = 1  # this walrus build allows one sem wait per instruction


def _split_multiwaits(nc, mybir):
    """Walrus here rejects instructions with >1 sem wait: move extra waits
    onto preceding NoOps on the same engine."""
    for bb in nc.main_func.blocks:
        new_list = []
        for ins in bb.instructions:
            si = ins.sync_info
            if si is not None and si.on_wait and len(si.on_wait) > _MAX_WAITS:
                waits = list(si.on_wait)
                si.on_wait[:] = waits[:_MAX_WAITS]
                for i in range(_MAX_WAITS, len(waits), _MAX_WAITS):
                    nop = mybir.InstNoOp(
                        name=f"I-splitw-{nc.next_id()}",
                        engine=ins.engine,
                        sync_info=mybir.SyncInfo(
                            on_wait=waits[i : i + _MAX_WAITS], on_update=[]
                        ),
                    )
                    nc.register_instruction(nop, overwrite=True)
                    new_list.append(nop)
            new_list.append(ins)
        bb.instructions[:] = new_list


def _patch_tile_drain(tile_mod, mybir):
    """Tail drain waits on the whole global clock; split to one wait/inst."""
    from concourse.vector_clock import ScopedClock

    def _drain_and_barrier_split(self, tick_clock, wait_clock):
        nc = self.nc
        carrier = nc.sync.nop(nofuse=True, hint="tile_drain_waits")
        wait_clock.add_sem_waits(
            carrier.ins, ScopedClock({None: tick_clock.global_clock})
        )
        waits = list(carrier.ins.sync_info.on_wait)
        if len(waits) > _MAX_WAITS:
            carrier.ins.sync_info.on_wait[:] = waits[:_MAX_WAITS]
            for i in range(_MAX_WAITS, len(waits), _MAX_WAITS):
                extra = nc.sync.nop(nofuse=True, hint="tile_drain_waits")
                extra.ins.sync_info = mybir.SyncInfo(
                    on_wait=waits[i : i + _MAX_WAITS], on_update=[]
                )
        nc.sync.drain()
        nc.all_engine_barrier()
        assert self.sems is not None
        popped = nc._tile_sem_poison_stack.pop()
        assert popped is self._sem_poison
        nc.clear_and_free_semaphores(list(self.sems.allocated().values()))
        nc.all_engine_barrier()

    tile_mod.TileContext._drain_and_barrier = _drain_and_barrier_split


_NC_CACHE = {}


def _build_nc():
    if "nc" in _NC_CACHE:
        return _NC_CACHE["nc"]
    import concourse.bass as bass
    import concourse.mybir as mybir
    import concourse.tile as tile

    _patch_tile_drain(tile, mybir)

    f32 = mybir.dt.float32
    f16 = mybir.dt.float16
    op = mybir.AluOpType
    af = mybir.ActivationFunctionType

    nc = bass.Bass()
    img = nc.declare_dram_parameter("image", [C, ROWS, W], f32, isOutput=False)
    tblp = nc.declare_dram_parameter(
        "tbl", [NBLK, 128, GC * GD * GW], f32, isOutput=False
    )
    zbp = nc.declare_dram_parameter("zbias", [128, GD], f32, isOutput=False)
    idp = nc.declare_dram_parameter("ident", [128, 128], f16, isOutput=False)
    pr0p = nc.declare_dram_parameter("prof0", [128, W], f16, isOutput=False)
    pr1p = nc.declare_dram_parameter("prof1", [128, W], f16, isOutput=False)
    outp = nc.declare_dram_parameter("out", [C, ROWS, W], f32, isOutput=True)

    def tidx(c, z, xs):
        return (c * GD + z) * GW + xs

    v = nc.vector
    g = nc.gpsimd

    # Per-(c,z,xs) matmul pieces: window split at the PSUM half boundary.
    HALF = W // 2

    def pieces(wa, wb):
        """Split [wa, wb) at col HALF -> [(half, lo, hi)] (absolute cols)."""
        out = []
        if wa < HALF:
            out.append((0, wa, min(wb, HALF)))
        if wb > HALF:
            out.append((1, max(wa, HALF), wb))
        return out

    with tile.TileContext(nc) as tc:
        with (
            tc.tile_pool(name="const", bufs=1) as cpool,
            tc.tile_pool(name="tbl", bufs=2) as tblpool,
            tc.tile_pool(name="img32", bufs=1) as i32pool,
            tc.tile_pool(name="img16", bufs=2) as i16pool,
            tc.tile_pool(name="uzp", bufs=2) as uzpool,
            tc.tile_pool(name="mz", bufs=1) as mzpool,
            tc.tile_pool(name="pp", bufs=1) as ppool,
            tc.tile_pool(name="diag", bufs=24) as dpool,
            tc.tile_pool(name="acc16", bufs=1) as apool,
            tc.tile_pool(name="outp", bufs=2) as opool,
            tc.tile_pool(name="psum", bufs=1, space="PSUM") as pspool,
        ):
            ident = cpool.tile([128, 128], f16, tag="ident")
            nc.sync.dma_start(ident[:], idp[:])
            zdiag = cpool.tile([128, 128], f16, tag="zdiag")
            g.memset(zdiag[:], 0.0)
            prof = []
            for k, pp_ in enumerate((pr0p, pr1p)):
                t = cpool.tile([128, W], f16, tag=f"prof{k}")
                nc.sync.dma_start(t[:], pp_[:])
                prof.append(t)
            zb_t = cpool.tile([128, GD], f32, tag="zbias")
            nc.sync.dma_start(zb_t[:], zbp[:])

            for blk in range(NBLK):
                rows = slice(blk * 128, (blk + 1) * 128)
                rgb = []
                for ch in range(C):
                    t = i32pool.tile([128, W], f32, tag=f"img{ch}")
                    nc.sync.dma_start(t[:], img[ch, rows, :])
                    rgb.append(t)
                tbl_t = tblpool.tile([128, GC * GD * GW], f32, tag="tbl")
                nc.sync.dma_start(tbl_t[:], tblp[blk])

                # uz = 15 * luminance (f32, DVE)
                tmp = uzpool.tile([128, W], f32, tag="uztmp")
                uz = uzpool.tile([128, W], f32, tag="uz")
                v.tensor_scalar_mul(tmp[:], rgb[0][:], 0.299 * 15.0)
                v.scalar_tensor_tensor(
                    uz[:], rgb[1][:], 0.587 * 15.0, tmp[:], op.mult, op.add
                )
                v.scalar_tensor_tensor(
                    tmp[:], rgb[2][:], 0.114 * 15.0, uz[:], op.mult, op.add
                )
                uz = tmp  # final uz

                # fp16 rgb for the apply (ACT)
                rgb16 = []
                for ch, src_t in enumerate(rgb):
                    t16 = i16pool.tile([128, W], f16, tag=f"img16_{ch}")
                    nc.scalar.copy(t16[:], src_t[:])
                    rgb16.append(t16)

                # tents mz[z] = relu(1 - |uz - z|)  (ACT, fp16 out)
                mz = []
                for z in range(GD):
                    d = uzpool.tile([128, W], f32, tag="mzd")
                    m = mzpool.tile([128, W], f16, tag=f"mz{z}")
                    nc.scalar.activation(d[:], uz[:], af.Abs, bias=zb_t[:, z : z + 1])
                    nc.scalar.activation(m[:], d[:], af.Relu, bias=1.0, scale=-1.0)
                    mz.append(m)

                # parity product tiles P0/P1 (DVE TT fp16)
                P = []
                for z in range(GD):
                    pz = []
                    for k in range(2):
                        t = ppool.tile([128, W], f16, tag=f"P{k}_{z}", name=f"P{k}_{z}")
                        v.tensor_tensor(t[:], mz[z][:], prof[k][:], op.mult)
                        pz.append(t)
                    P.append(pz)

                def PA(z, i):  # tap-A holder on interval i
                    return P[z][i % 2]

                def PB(z, i):  # tap-B holder on interval i
                    return P[z][1 - i % 2]

                def mk_diag(c, z, xs):
                    d = dpool.tile([128, 128], f16, tag="diag", name="diag")
                    eng = g if xs % _POOL_DIAG_MOD == 0 else v
                    eng.tensor_scalar_mul(
                        d[:], ident[:], tbl_t[:, tidx(c, z, xs) : tidx(c, z, xs) + 1]
                    )
                    return d

                # accumulate coefficient planes on PE
                acc16 = [
                    apool.tile([128, W], f16, tag=f"acc16_{c}", name=f"acc16_{c}") for c in range(GC)
                ]
                for cg in range(0, GC, 3):
                    for c in range(cg, cg + 3):
                        acc = [
                            pspool.tile(
                                [128, HALF], f32, tag=f"ps{c % 3}_{h}",
                                name=f"ps{c % 3}_{h}", space="PSUM"
                            )
                            for h in range(2)
                        ]

                        def mm(d, src, wa, wb, start, stop):
                            for h, lo, hi in pieces(wa, wb):
                                nc.tensor.matmul(
                                    acc[h][:, lo - h * HALF : hi - h * HALF],
                                    d[:],
                                    src[:, lo:hi],
                                    start=start,
                                    stop=stop,
                                )

                        # open each half-bank with a full-width zeroing matmul
                        # (hw start=True resets the whole bank; make that
                        # deterministic under either reset semantics)
                        for h in range(2):
                            nc.tensor.matmul(
                                acc[h][:],
                                zdiag[:],
                                prof[0][:, h * HALF : (h + 1) * HALF],
                                start=True,
                                stop=False,
                            )
                        # z = 0: per-interval tap pairs (accumulate onto 0)
                        d0 = [mk_diag(c, 0, xs) for xs in range(GW)]
                        for i in range(NI):
                            ia, ib = _BOUNDS[i]
                            mm(d0[i], PA(0, i), ia, ib, False, False)
                            mm(d0[i + 1], PB(0, i), ia, ib, False, False)
                        # z = 1..14: one window matmul per xs (parity tile)
                        for z in range(1, GD - 1):
                            for xs in range(GW):
                                wa, wb = _window(xs)
                                mm(mk_diag(c, z, xs), P[z][xs % 2], wa, wb, False, False)
                        # z = 15: per-interval tap pairs, last touch stops
                        zl = GD - 1
                        dl = [mk_diag(c, zl, xs) for xs in range(GW)]
                        for i in range(NI):
                            ia, ib = _BOUNDS[i]
                            mm(dl[i], PA(zl, i), ia, ib, False, False)
                            mm(dl[i + 1], PB(zl, i), ia, ib, False, True)

                        # evict PSUM -> SBUF fp16 (ACT)
                        for h in range(2):
                            nc.scalar.copy(
                                acc16[c][:, h * HALF : (h + 1) * HALF], acc[h][:]
                            )

                # apply: out_o = clip(acc.A @ rgb + bias)  (DVE)
                r16, g16, b16 = rgb16
                for o in range(C):
                    p1 = opool.tile([128, W], f16, tag="p1")
                    p2 = opool.tile([128, W], f16, tag="p2")
                    v.tensor_tensor(p1[:], acc16[3 * o][:], r16[:], op.mult)
                    v.tensor_tensor(p2[:], acc16[3 * o + 1][:], g16[:], op.mult)
                    v.tensor_tensor(p1[:], p1[:], p2[:], op.add)
                    v.tensor_tensor(p2[:], acc16[3 * o + 2][:], b16[:], op.mult)
                    v.tensor_tensor(p1[:], p1[:], p2[:], op.add)
                    v.tensor_tensor(p1[:], p1[:], acc16[9 + o][:], op.add)
                    ot = opool.tile([128, W], f32, tag="ot")
                    v.tensor_scalar_max(p1[:], p1[:], 0.0)
                    v.tensor_scalar_min(ot[:], p1[:], 1.0)
                    nc.sync.dma_start(outp[o, rows, :], ot[:])

    _split_multiwaits(nc, mybir)
    _NC_CACHE["nc"] = nc
    return nc


# ---------------------------------------------------------------------------
# Public entry point
# ---------------------------------------------------------------------------


_TBL_CACHE = {}


def kernel(grid: np.ndarray, image: np.ndarray) -> np.ndarray:
    from concourse.bass_utils import run_bass_kernel_spmd

    grid = np.asarray(grid, dtype=np.float32)
    image = np.asarray(image, dtype=np.float32)

    nc = _build_nc()
    zbias = _host_zbias()
    ident = _host_identity()
    prof0, prof1 = _host_profiles()
    gkey = hash(grid.tobytes())
    in_maps = []
    for core in range(NCORES):
        b, half = core // 2, core % 2
        slab = np.ascontiguousarray(image[b][:, half * ROWS : (half + 1) * ROWS, :])
        tk = (gkey, core)
        if tk not in _TBL_CACHE:
            _TBL_CACHE[tk] = _host_tables(grid[b], half)
        in_maps.append(
            {
                "image": slab,
                "tbl": _TBL_CACHE[tk],
                "zbias": zbias,
                "ident": ident,
                "prof0": prof0,
                "prof1": prof1,
            }
        )

    res = run_bass_kernel_spmd(nc, in_maps, list(range(NCORES)))

    out = np.empty((B, C, H, W), np.float32)
    for core in range(NCORES):
        b, half = core // 2, core % 2
        out[b][:, half * ROWS : (half + 1) * ROWS, :] = res.results[core]["out"]
    return out


# revision 16
# speedup vs baseline: 1.0165x; 1.0165x over previous
"""BilateralGrid (HDRNet slicing) Trainium2 Bass kernel — PE-accumulate edition.

Full inputs -> full output. Sharding: 8 cores = (batch b, H-half); each core
processes an image slab (3, 512, 1024) of one batch.

Per 128-row block the slicing is computed as:

  uz      = 15 * luminance(R, G, B)                       (DVE, f32)
  mz[z]   = relu(1 - |uz - z|)            z = 0..15       (ACT tents, fp16)
  P0/P1   = mz * static x-blend parity profiles           (DVE TT, fp16)
  acc_c  += diag(t[c,z,xs]) @ P_par(xs)[:, window(xs)]    (PE matmuls, PSUM f32)
  acc16_c = copy(acc_c)                                   (ACT, PSUM->SBUF fp16)
  out_o   = clip(acc_{3o}*R + acc_{3o+1}*G + acc_{3o+2}*B + acc_{9+o}, 0, 1)

The x interpolation is folded into the matmul stream: every output column lies
in exactly one x-interval i (between grid columns i and i+1) and receives
tap-A weight (1-fx) toward xs=i and tap-B weight fx toward xs=i+1.  Writing
the tap weights into two static parity profiles (prof0 holds (1-fx) on even
intervals / fx on odd, prof1 the complement) makes the two taps sharing the
per-partition table scalar t[c,z,xs] — tap-A of interval xs and tap-B of
interval xs-1 — occupy ONE contiguous column window with ONE parity tile, so
each (c,z,xs) is a single PSUM-accumulating matmul with stationary
diag(t[c,z,xs]).  Diagonals are built on the fly from a fp16 identity via
one tensor_scalar_mul each.  t[row, c, z, xs] is the y-interpolated grid
table per image row, built on host from the tiny grid input.
"""

import numpy as np

B, C, H, W = 4, 3, 1024, 1024
GD, GH, GW, GC = 16, 16, 8, 12  # grid z, y, x extents; coeff channels
NCORES = 8
ROWS = H // 2  # rows per core
NBLK = ROWS // 128
NI = GW - 1  # 7 x-intervals

_POOL_DIAG_MOD = 4  # xs % _POOL_DIAG_MOD == 0 -> diag built on gpsimd


def _intervals():
    """Interval bounds s_i..e_i (cols with min(floor(ux),6)==i) and fx."""
    ux = np.arange(W) * (GW - 1) / (W - 1.0)
    x0 = np.minimum(np.floor(ux).astype(np.int64), NI - 1)
    bounds = []
    for i in range(NI):
        idx = np.nonzero(x0 == i)[0]
        bounds.append((int(idx[0]), int(idx[-1]) + 1))
    fx = (ux - x0).astype(np.float32)  # in [0,1]; ==1 only at col W-1
    return fx, x0, bounds


_FX, _X0, _BOUNDS = _intervals()


def _window(xs):
    """Column window of the single matmul for tap-pair xs: union of tap-B on
    interval xs-1 and tap-A on interval xs."""
    wa = _BOUNDS[xs - 1][0] if xs > 0 else _BOUNDS[0][0]
    wb = _BOUNDS[xs][1] if xs < NI else _BOUNDS[NI - 1][1]
    return wa, wb


def _host_profiles():
    """prof0/prof1 [128, W] fp16: parity-interleaved (1-fx)/fx profiles."""
    p0 = np.where(_X0 % 2 == 0, 1.0 - _FX, _FX).astype(np.float16)
    p1 = np.where(_X0 % 2 == 0, _FX, 1.0 - _FX).astype(np.float16)
    return (
        np.ascontiguousarray(np.tile(p0, (128, 1))),
        np.ascontiguousarray(np.tile(p1, (128, 1))),
    )


def _host_tables(grid_b, half):
    """t[row, c, z, xs] for this core's 512 rows -> (NBLK, 128, 1536) f32."""
    h = half * ROWS + np.arange(ROWS)
    uy = h * (GH - 1) / (H - 1.0)
    y0 = np.minimum(np.floor(uy).astype(np.int64), GH - 2)
    fy = (uy - y0).astype(np.float32)
    gy0 = grid_b[:, :, y0, :]  # (12, 16, 512, 8)
    gy1 = grid_b[:, :, y0 + 1, :]
    tbl = (1 - fy)[None, None, :, None] * gy0 + fy[None, None, :, None] * gy1
    tbl = np.transpose(tbl, (2, 0, 1, 3))  # (512, c, z, xs)
    return np.ascontiguousarray(
        tbl.reshape(NBLK, 128, GC * GD * GW).astype(np.float32)
    )


def _host_zbias():
    """bias column per z: -z, replicated over partitions -> (128, 16)."""
    return np.tile(-np.arange(GD, dtype=np.float32), (128, 1))


def _host_identity():
    return np.eye(128, dtype=np.float16)


# ---------------------------------------------------------------------------
# Bass program
# ---------------------------------------------------------------------------

_MAX_WAITS = 1  # this walrus build allows one sem wait per instruction


def _split_multiwaits(nc, mybir):
    """Walrus here rejects instructions with >1 sem wait: move extra waits
    onto preceding NoOps on the same engine."""
    for bb in nc.main_func.blocks:
        new_list = []
        for ins in bb.instructions:
            si = ins.sync_info
            if si is not None and si.on_wait and len(si.on_wait) > _MAX_WAITS:
                waits = list(si.on_wait)
                si.on_wait[:] = waits[:_MAX_WAITS]
                for i in range(_MAX_WAITS, len(waits), _MAX_WAITS):
                    nop = mybir.InstNoOp(
                        name=f"I-splitw-{nc.next_id()}",
                        engine=ins.engine,
                        sync_info=mybir.SyncInfo(
                            on_wait=waits[i : i + _MAX_WAITS], on_update=[]
                        ),
                    )
                    nc.register_instruction(nop, overwrite=True)
                    new_list.append(nop)
            new_list.append(ins)
        bb.instructions[:] = new_list


def _patch_tile_drain(tile_mod, mybir):
    """Tail drain waits on the whole global clock; split to one wait/inst."""
    from concourse.vector_clock import ScopedClock

    def _drain_and_barrier_split(self, tick_clock, wait_clock):
        nc = self.nc
        carrier = nc.sync.nop(nofuse=True, hint="tile_drain_waits")
        wait_clock.add_sem_waits(
            carrier.ins, ScopedClock({None: tick_clock.global_clock})
        )
        waits = list(carrier.ins.sync_info.on_wait)
        if len(waits) > _MAX_WAITS:
            carrier.ins.sync_info.on_wait[:] = waits[:_MAX_WAITS]
            for i in range(_MAX_WAITS, len(waits), _MAX_WAITS):
                extra = nc.sync.nop(nofuse=True, hint="tile_drain_waits")
                extra.ins.sync_info = mybir.SyncInfo(
                    on_wait=waits[i : i + _MAX_WAITS], on_update=[]
                )
        nc.sync.drain()
        nc.all_engine_barrier()
        assert self.sems is not None
        popped = nc._tile_sem_poison_stack.pop()
        assert popped is self._sem_poison
        nc.clear_and_free_semaphores(list(self.sems.allocated().values()))
        nc.all_engine_barrier()

    tile_mod.TileContext._drain_and_barrier = _drain_and_barrier_split


_NC_CACHE = {}


def _build_nc():
    if "nc" in _NC_CACHE:
        return _NC_CACHE["nc"]
    import concourse.bass as bass
    import concourse.mybir as mybir
    import concourse.tile as tile

    _patch_tile_drain(tile, mybir)

    f32 = mybir.dt.float32
    f16 = mybir.dt.float16
    op = mybir.AluOpType
    af = mybir.ActivationFunctionType

    nc = bass.Bass()
    img = nc.declare_dram_parameter("image", [C, ROWS, W], f32, isOutput=False)
    tblp = nc.declare_dram_parameter(
        "tbl", [NBLK, 128, GC * GD * GW], f32, isOutput=False
    )
    zbp = nc.declare_dram_parameter("zbias", [128, GD], f32, isOutput=False)
    idp = nc.declare_dram_parameter("ident", [128, 128], f16, isOutput=False)
    pr0p = nc.declare_dram_parameter("prof0", [128, W], f16, isOutput=False)
    pr1p = nc.declare_dram_parameter("prof1", [128, W], f16, isOutput=False)
    outp = nc.declare_dram_parameter("out", [C, ROWS, W], f32, isOutput=True)

    def tidx(c, z, xs):
        return (c * GD + z) * GW + xs

    v = nc.vector
    g = nc.gpsimd

    # Per-(c,z,xs) matmul pieces: window split at the PSUM half boundary.
    HALF = W // 2

    def pieces(wa, wb):
        """Split [wa, wb) at col HALF -> [(half, lo, hi)] (absolute cols)."""
        out = []
        if wa < HALF:
            out.append((0, wa, min(wb, HALF)))
        if wb > HALF:
            out.append((1, max(wa, HALF), wb))
        return out

    with tile.TileContext(nc) as tc:
        with (
            tc.tile_pool(name="const", bufs=1) as cpool,
            tc.tile_pool(name="tbl", bufs=2) as tblpool,
            tc.tile_pool(name="img32", bufs=1) as i32pool,
            tc.tile_pool(name="img16", bufs=2) as i16pool,
            tc.tile_pool(name="uzp", bufs=2) as uzpool,
            tc.tile_pool(name="mz", bufs=1) as mzpool,
            tc.tile_pool(name="pp", bufs=1) as ppool,
            tc.tile_pool(name="diag", bufs=24) as dpool,
            tc.tile_pool(name="acc16", bufs=1) as apool,
            tc.tile_pool(name="outp", bufs=2) as opool,
            tc.tile_pool(name="psum", bufs=1, space="PSUM") as pspool,
        ):
            ident = cpool.tile([128, 128], f16, tag="ident")
            nc.sync.dma_start(ident[:], idp[:])
            prof = []
            for k, pp_ in enumerate((pr0p, pr1p)):
                t = cpool.tile([128, W], f16, tag=f"prof{k}")
                nc.sync.dma_start(t[:], pp_[:])
                prof.append(t)
            zb_t = cpool.tile([128, GD], f32, tag="zbias")
            nc.sync.dma_start(zb_t[:], zbp[:])

            # ---- software-pipelined emission -------------------------
            # Per-block stages are emitted so that each engine's in-order
            # stream never blocks the PE: next block's uz comes before this
            # block's diag stream (DVE), next block's tents are chunked
            # between this block's evictions (ACT), and next block's P
            # products are emitted right after the last channel's z-level
            # frees the P[z] buffers (DVE).
            state = {}

            def emit_dma_uz(blk):
                rows = slice(blk * 128, (blk + 1) * 128)
                st = {"rows": rows}
                rgb = []
                for ch in range(C):
                    t = i32pool.tile([128, W], f32, tag=f"img{ch}", name=f"img{ch}")
                    nc.sync.dma_start(t[:], img[ch, rows, :])
                    rgb.append(t)
                tbl_t = tblpool.tile([128, GC * GD * GW], f32, tag="tbl")
                st["tbl"] = tbl_t
                nc.sync.dma_start(tbl_t[:], tblp[blk])

                # uz = 15 * luminance (f32, DVE)
                tmp = uzpool.tile([128, W], f32, tag="uztmp")
                uzt = uzpool.tile([128, W], f32, tag="uz")
                v.tensor_scalar_mul(tmp[:], rgb[0][:], 0.299 * 15.0)
                v.scalar_tensor_tensor(
                    uzt[:], rgb[1][:], 0.587 * 15.0, tmp[:], op.mult, op.add
                )
                v.scalar_tensor_tensor(
                    tmp[:], rgb[2][:], 0.114 * 15.0, uzt[:], op.mult, op.add
                )
                st["uz"] = tmp

                # fp16 rgb for the apply (ACT)
                rgb16 = []
                for ch, src_t in enumerate(rgb):
                    t16 = i16pool.tile([128, W], f16, tag=f"img16_{ch}", name=f"i16_{ch}")
                    nc.scalar.copy(t16[:], src_t[:])
                    rgb16.append(t16)
                st["rgb16"] = rgb16
                st["mz"] = [None] * GD
                st["P"] = [None] * GD
                state[blk] = st
                return st

            def emit_tents(blk, zs):
                """tents mz[z] = relu(1 - |uz - z|) (ACT, fp16 out)."""
                st = state[blk]
                for z in zs:
                    d = uzpool.tile([128, W], f32, tag="mzd")
                    m = mzpool.tile([128, W], f16, tag=f"mz{z}", name=f"mz{z}")
                    nc.scalar.activation(
                        d[:], st["uz"][:], af.Abs, bias=zb_t[:, z : z + 1]
                    )
                    nc.scalar.activation(m[:], d[:], af.Relu, bias=1.0, scale=-1.0)
                    st["mz"][z] = m

            def emit_P(blk, z):
                """parity product tiles P0/P1 for one z (DVE TT fp16)."""
                st = state[blk]
                pz = []
                for k in range(2):
                    t = ppool.tile([128, W], f16, tag=f"P{k}_{z}", name=f"P{k}_{z}")
                    v.tensor_tensor(t[:], st["mz"][z][:], prof[k][:], op.mult)
                    pz.append(t)
                st["P"][z] = pz

            def emit_sweep(blk, hook_z, hook_evict):
                """PE accumulation sweep. hook_z(z) fires after the last
                channel's z-level (P[z] freed); hook_evict(c) after each
                eviction (ACT slack point)."""
                st = state[blk]
                tbl_t = st["tbl"]
                P = st["P"]

                def PA(z, i):  # tap-A holder on interval i
                    return P[z][i % 2]

                def PB(z, i):  # tap-B holder on interval i
                    return P[z][1 - i % 2]

                def mk_diag(c, z, xs):
                    d = dpool.tile([128, 128], f16, tag="diag", name="diag")
                    eng = g if xs % _POOL_DIAG_MOD == 0 else v
                    eng.tensor_scalar_mul(
                        d[:], ident[:], tbl_t[:, tidx(c, z, xs) : tidx(c, z, xs) + 1]
                    )
                    return d

                acc16 = [
                    apool.tile([128, W], f16, tag=f"acc16_{c}", name=f"acc16_{c}")
                    for c in range(GC)
                ]
                st["acc16"] = acc16
                for c in range(GC):
                    acc = [
                        pspool.tile(
                            [128, HALF], f32, tag=f"ps{c % 4}_{h}",
                            name=f"ps{c % 4}_{h}", space="PSUM"
                        )
                        for h in range(2)
                    ]

                    def mm(d, src, wa, wb, start, stop):
                        for h, lo, hi in pieces(wa, wb):
                            nc.tensor.matmul(
                                acc[h][:, lo - h * HALF : hi - h * HALF],
                                d[:],
                                src[:, lo:hi],
                                start=start,
                                stop=stop,
                            )

                    # open each half-bank zeroed (hw start=True resets the
                    # whole bank, so a partial-width start is unsafe; a
                    # full-width zero-write off the PE is deterministic; gpsimd
                    # cannot write PSUM here, so zero via ACT: relu(0*x+0)
                    for h in range(2):
                        nc.scalar.activation(
                            acc[h][:], prof[0][:, :HALF], af.Relu,
                            bias=0.0, scale=0.0,
                        )
                    # z = 0: per-interval tap pairs (accumulate onto 0)
                    d0 = [mk_diag(c, 0, xs) for xs in range(GW)]
                    for i in range(NI):
                        ia, ib = _BOUNDS[i]
                        mm(d0[i], PA(0, i), ia, ib, False, False)
                        mm(d0[i + 1], PB(0, i), ia, ib, False, False)
                    if c == GC - 1 and hook_z is not None:
                        hook_z(0)
                    # z = 1..14: one window matmul per xs (parity tile)
                    for z in range(1, GD - 1):
                        for xs in range(GW):
                            wa, wb = _window(xs)
                            mm(mk_diag(c, z, xs), P[z][xs % 2], wa, wb, False, False)
                        if c == GC - 1 and hook_z is not None:
                            hook_z(z)
                    # z = 15: per-interval tap pairs, last touch stops
                    zl = GD - 1
                    dl = [mk_diag(c, zl, xs) for xs in range(GW)]
                    for i in range(NI):
                        ia, ib = _BOUNDS[i]
                        mm(dl[i], PA(zl, i), ia, ib, False, False)
                        mm(dl[i + 1], PB(zl, i), ia, ib, False, True)
                    if c == GC - 1 and hook_z is not None:
                        hook_z(zl)

                    # evict PSUM -> SBUF fp16 (ACT)
                    for h in range(2):
                        nc.scalar.copy(
                            acc16[c][:, h * HALF : (h + 1) * HALF], acc[h][:]
                        )
                    if hook_evict is not None:
                        hook_evict(c)

            def emit_apply(blk):
                """out_o = clip(acc.A @ rgb + bias)  (DVE)."""
                st = state[blk]
                acc16 = st["acc16"]
                r16, g16, b16 = st["rgb16"]
                rows = st["rows"]
                for o in range(C):
                    p1 = opool.tile([128, W], f16, tag="p1")
                    p2 = opool.tile([128, W], f16, tag="p2")
                    v.tensor_tensor(p1[:], acc16[3 * o][:], r16[:], op.mult)
                    v.tensor_tensor(p2[:], acc16[3 * o + 1][:], g16[:], op.mult)
                    v.tensor_tensor(p1[:], p1[:], p2[:], op.add)
                    v.tensor_tensor(p2[:], acc16[3 * o + 2][:], b16[:], op.mult)
                    v.tensor_tensor(p1[:], p1[:], p2[:], op.add)
                    v.tensor_tensor(p1[:], p1[:], acc16[9 + o][:], op.add)
                    ot = opool.tile([128, W], f32, tag="ot")
                    v.tensor_scalar_max(p1[:], p1[:], 0.0)
                    v.tensor_scalar_min(ot[:], p1[:], 1.0)
                    nc.sync.dma_start(outp[o, rows, :], ot[:])
                del state[blk]

            emit_dma_uz(0)
            emit_tents(0, range(GD))
            for z in range(GD):
                emit_P(0, z)
            for blk in range(NBLK):
                nxt = blk + 1 if blk + 1 < NBLK else None
                if nxt is not None:
                    emit_dma_uz(nxt)

                    def hook_z(z, nxt=nxt):
                        emit_P(nxt, z)

                    def hook_evict(c, nxt=nxt):
                        # 4 tent chunks spread over this block's evictions,
                        # all before c=11's sweep needs them for emit_P
                        if c % 3 == 1:
                            k = c // 3
                            emit_tents(nxt, range(k * 4, k * 4 + 4))
                else:
                    hook_z = hook_evict = None
                emit_sweep(blk, hook_z, hook_evict)
                emit_apply(blk)

    _split_multiwaits(nc, mybir)
    _NC_CACHE["nc"] = nc
    return nc


# ---------------------------------------------------------------------------
# Public entry point
# ---------------------------------------------------------------------------


_TBL_CACHE = {}


def kernel(grid: np.ndarray, image: np.ndarray) -> np.ndarray:
    from concourse.bass_utils import run_bass_kernel_spmd

    grid = np.asarray(grid, dtype=np.float32)
    image = np.asarray(image, dtype=np.float32)

    nc = _build_nc()
    zbias = _host_zbias()
    ident = _host_identity()
    prof0, prof1 = _host_profiles()
    gkey = hash(grid.tobytes())
    in_maps = []
    for core in range(NCORES):
        b, half = core // 2, core % 2
        slab = np.ascontiguousarray(image[b][:, half * ROWS : (half + 1) * ROWS, :])
        tk = (gkey, core)
        if tk not in _TBL_CACHE:
            _TBL_CACHE[tk] = _host_tables(grid[b], half)
        in_maps.append(
            {
                "image": slab,
                "tbl": _TBL_CACHE[tk],
                "zbias": zbias,
                "ident": ident,
                "prof0": prof0,
                "prof1": prof1,
            }
        )

    res = run_bass_kernel_spmd(nc, in_maps, list(range(NCORES)))

    out = np.empty((B, C, H, W), np.float32)
    for core in range(NCORES):
        b, half = core // 2, core % 2
        out[b][:, half * ROWS : (half + 1) * ROWS, :] = res.results[core]["out"]
    return out
